# revision 8
# baseline (speedup 1.0000x reference)
"""Trainium2 Bass kernel for nn_CPADConvOffsetStage.

The reference module is:
  up_posi = grid_sample_bilinear_border(posi_map -> [B,16,GP,GP], grid = base + offset*scale)
  h       = relu(w1 @ up_posi + b1)           (1x1 conv)
  weights = (w2 @ h + b2).reshape(B,64,9,H,W) (1x1 conv -> per-pixel 3x3 kernels)
  x_adapt = w_ca @ x                          (1x1 conv)
  out     = sum_k weights[:,:,k] * unfold3x3(x_adapt)[:,:,k] + bias

In setup_inputs() posi_map is spatially constant per channel (jnp.ones).
Bilinear interpolation weights sum to exactly 1, so up_posi is spatially
constant => h, weights are spatially constant => the whole module reduces
to ONE dense 3x3 convolution with host-precomputable weights
    Wfull[o,c,k] = wk[o,k] * w_ca[o,c],   wk = (w2 @ relu(w1 @ v + b1) + b2)
plus the bias.  The kernel below runs that conv data-parallel over batch
(1 batch image per NeuronCore, 8 cores).

Device kernel ("quad" variant): the image is split into top/bottom
halves living on SBUF partitions 0:64 / 64:128.  Each 3x3 tap is a
K=64 matmul addressed to one 64x64 quadrant of the PE array via
tile_position=(row, col): row group = image half (where the rhs data
lives), col group = which 64-partition half of the PSUM bank the
output tile goes to.  Four quadrant matmuls run concurrently, so the
PE streams at the full 128x128 capacity bound even though K=64.
Each PSUM bank is evacuated by a single scalar-engine activation
(bias fused, bf16 output) -- the vector engine is not used at all.
A block of dependency-free warm-up matmuls runs while the first
input DMA is in flight so the PE HAM clock-gate is already released
(2.4 GHz) when real work arrives.

If posi_map is NOT per-channel spatially constant (never the case for the
shipped setup_inputs), we fall back to an exact numpy port of the
reference.
"""

import os
import numpy as np
from contextlib import ExitStack

import concourse.bass as bass
import concourse.tile as tile
from concourse import mybir
from concourse.bass_utils import run_bass_kernel_spmd

# Problem constants (hardcoded per contract)
B, C, H, W = 8, 64, 128, 128
OC = 64
KK = 3
POSI_CH, GP = 16, 16
NCORES = 8
F32 = mybir.dt.float32
BF16 = mybir.dt.bfloat16

HPAD, WPAD = H + 2, W + 2      # host-padded image (130 x 130)
HH = H // 2                    # rows per image half (64)
HHPAD = HH + 2                 # padded rows per half (66)
NITER = HH // 8                # 8-row iterations per half

_cached_nc = None
_cached_variant = None
last_results = None            # test harness introspection


def _ensure_ntff_hook():
    """Register the axon NTFF-profile hook that this image's antenv lacks.

    run_bass_kernel_spmd(trace=True) under axon needs
    antenv.axon_hooks.get_axon_ntff_profile_hook; the hook machinery
    exists in trn_agent_boot but was never registered because
    antenv.axon_hooks is missing.  Recreate the module in sys.modules.
    """
    import sys
    import types

    if "antenv.axon_hooks" in sys.modules:
        return
    try:
        from trn_agent_boot.trn_boot import _ntff_profile_via_ctypes

        hook = _ntff_profile_via_ctypes("/opt/axon/libaxon_pjrt.so")
    except Exception:
        hook = None
    mod = types.ModuleType("antenv.axon_hooks")
    mod.get_axon_ntff_profile_hook = lambda: hook
    mod.set_axon_ntff_profile_hook = lambda h: None
    sys.modules["antenv.axon_hooks"] = mod
    try:
        import antenv

        antenv.axon_hooks = mod
    except Exception:
        pass


def _build_conv_nc_quad(warm_mms=12, split_waits=True):
    """3x3 conv, 64->64 ch, on one host-padded [64,130,130] bf16 image.

    SPMD over 8 cores, one batch image per core.  Output DRAM layout is
    [half, p, iter, r, w] (p = 4-row-subtile * 64 + channel) so every
    output DMA is one partition-contiguous 2 KB run per partition; the
    host un-permutes.
    """
    nc = bass.Bass()
    x_d = nc.declare_dram_parameter("x", [C, HPAD, WPAD], BF16, isOutput=False)
    w_d = nc.declare_dram_parameter("wts", [128, 9 * OC], BF16, isOutput=False)
    b_d = nc.declare_dram_parameter("wb", [128, 1], F32, isOutput=False)
    o_d = nc.declare_dram_parameter(
        "out", [2, 128, NITER, 4, W], BF16, isOutput=True
    )

    with ExitStack() as ctx:
        tc = ctx.enter_context(tile.TileContext(nc))
        singles = ctx.enter_context(tc.tile_pool(name="singles", bufs=1))
        outs = ctx.enter_context(tc.tile_pool(name="outs", bufs=2))
        psum = ctx.enter_context(tc.tile_pool(name="psum", bufs=6, space="PSUM"))
        warmp = ctx.enter_context(tc.tile_pool(name="warmp", bufs=1, space="PSUM"))

        # ---- PE warm-up: dependency-free matmuls on a memset region ----
        # They run while the first input DMA is in flight, releasing the
        # HAM clock gate (1.2 -> 2.4 GHz) before real matmuls start.
        wm = singles.tile([128, 512], BF16, name="wm")
        nc.gpsimd.memset(wm[:, :], 0.0)
        wps = warmp.tile([128, 512], F32, name="wps")
        for _ in range(warm_mms):
            nc.tensor.matmul(
                wps[:, :], lhsT=wm[:, 0:128], rhs=wm[:, :],
                start=True, stop=True, skip_group_check=True,
            )

        # ---- parameters + input (two HWDGE queues issue in parallel) ----
        w_sb = singles.tile([128, 9 * OC], BF16, name="w_sb")
        b_sb = singles.tile([128, 1], F32, name="b_sb")
        # xb: partitions 0:64 = padded rows 0:66 (top half),
        #     partitions 64:128 = padded rows 64:130 (bottom half).
        # Local row index r maps to padded row r (top) / 64+r (bottom).
        xb = singles.tile([128, HHPAD, WPAD], BF16, name="xb")
        chunks = [(0, 18), (18, 34), (34, 50), (50, 66)]
        with tc.high_priority():
            # wts/wb on the scalar HWDGE ring (tiny, parallel); ALL image
            # chunks serialized on the sync ring so chunk 0 gets the full
            # HBM bandwidth and lands as early as possible.
            nc.scalar.dma_start(out=w_sb[:, :], in_=w_d[:, :])
            nc.scalar.dma_start(out=b_sb[:, :], in_=b_d[:, :])
            for r0, r1 in chunks:
                nc.sync.dma_start(out=xb[0:C, r0:r1, :], in_=x_d[:, r0:r1, :])
                nc.sync.dma_start(
                    out=xb[C:128, r0:r1, :], in_=x_d[:, HH + r0 : HH + r1, :]
                )

        # ---- main loop: 8 iterations x (8 top rows + 8 bottom rows) ----
        for j in range(NITER):
            a = 8 * j                  # local row base of this iteration
            ps_t = psum.tile([128, 512], F32, tag="ps", name="ps_t")
            ps_b = psum.tile([128, 512], F32, tag="ps", name="ps_b")
            for t in range(9):
                di, dj = divmod(t, 3)
                st, sp = (t == 0), (t == 8)
                for half, ps in ((0, ps_t), (1, ps_b)):
                    pb = C * half
                    lhsT = w_sb[pb : pb + C, t * OC : (t + 1) * OC]
                    for cg in (0, 64):
                        rr = a + (cg // 64) * 4 + di
                        nc.tensor.matmul(
                            ps[cg : cg + OC, :],
                            lhsT=lhsT,
                            rhs=xb[pb : pb + C, rr : rr + 4, dj : dj + W],
                            start=st,
                            stop=sp,
                            tile_position=(pb, cg),
                            skip_group_check=True,
                        )
            if j % 2 == 0:
                o_top = outs.tile([128, 1024], BF16, tag="o_top", name="o_top")
                o_bot = outs.tile([128, 1024], BF16, tag="o_bot", name="o_bot")
            off = (j % 2) * 512
            # Top evac on ScalarE, bottom on VectorE: the two chains (and
            # their output DMAs, on separate HWDGE queues) run in parallel,
            # which matters most for the kernel tail.
            nc.scalar.activation(
                out=o_top[:, off : off + 512], in_=ps_t[:, :],
                func=mybir.ActivationFunctionType.Identity,
                bias=b_sb[:, 0:1], scale=1.0,
            )
            nc.vector.tensor_scalar_add(
                out=o_bot[:, off : off + 512], in0=ps_b[:, :],
                scalar1=b_sb[:, 0:1],
            )
            if j % 2 == 1:
                jj = j // 2
                nc.sync.dma_start(
                    out=o_d[0, :, 2 * jj : 2 * jj + 2, :, :],
                    in_=o_top[:, :].rearrange("p (s r w) -> p s r w", s=2, r=4),
                )
                nc.scalar.dma_start(
                    out=o_d[1, :, 2 * jj : 2 * jj + 2, :, :],
                    in_=o_bot[:, :].rearrange("p (s r w) -> p s r w", s=2, r=4),
                )
    if split_waits:
        _split_sync_waits(nc)
    return nc


def _split_sync_waits(nc, limit=1):
    """Hoist extra sync waits onto injected wait-only EventSemaphore ops.

    The neuronxcc walrus used under axon rejects compute instructions
    carrying more than one sync wait ("Too many sync wait commands", e.g.
    S3_LW / S3D3_AC structs).  Tile's sem assignment emits up to ~3.
    For every instruction with >limit waits, keep the first `limit` and
    prepend one wait-only EventSemaphore per extra wait on the same
    engine (same program position => same semantics).
    """
    import copy as _copy

    f = nc.m.functions[0]
    template = None
    for blk in f.blocks:
        for inst in blk.instructions:
            if type(inst).__name__ == "InstEventSemaphore":
                template = inst
                break
        if template is not None:
            break
    if template is None:
        return
    n_split = 0
    for blk in f.blocks:
        new_list = []
        changed = False
        for inst in blk.instructions:
            si = getattr(inst, "sync_info", None)
            waits = list(si.on_wait) if (si and si.on_wait) else []
            if len(waits) > limit:
                for w in waits[limit:]:
                    ev = _copy.deepcopy(template)
                    ev.name = f"waitsplit_{n_split}"
                    n_split += 1
                    ev.engine = inst.engine
                    ev.sync_info = mybir.SyncInfo(on_wait=[w], on_update=[])
                    new_list.append(ev)
                inst.sync_info = mybir.SyncInfo(
                    on_wait=waits[:limit], on_update=list(si.on_update or [])
                )
                changed = True
            new_list.append(inst)
        if changed:
            blk.instructions = new_list


def _host_conv_weights(posi_map, w1, b1, w2, b2, w_ca, bias):
    """Collapse the constant-posi_map weight generator on the host."""
    pm = np.asarray(posi_map, np.float64)[0]              # [16, GP, GP]
    vvec = pm.reshape(POSI_CH, -1)[:, 0]                  # per-channel constant
    h = np.maximum(np.asarray(w1, np.float64) @ vvec + np.asarray(b1, np.float64), 0.0)
    wvec = np.asarray(w2, np.float64) @ h + np.asarray(b2, np.float64)   # [576]
    wk = wvec.reshape(OC, 9)                              # [o, k]
    wca = np.asarray(w_ca, np.float64)                    # [o, c]
    wfull = wk[:, None, :] * wca[:, :, None]              # [o, c, k]
    return wfull, np.asarray(bias, np.float32).reshape(OC)


def _pack_quad(wfull, bias_vec):
    """lhsT layout for the quad kernel: [128, 9*64] with each tap's
    [c, o] block duplicated on both partition halves, plus [128,1] bias."""
    wts_half = np.ascontiguousarray(
        wfull.transpose(1, 2, 0).reshape(C, 9 * OC).astype(np.float32)
    )                                                     # [c, t*OC + o]
    wts = np.concatenate([wts_half, wts_half], axis=0)    # [128, 576]
    wb = np.concatenate([bias_vec, bias_vec]).reshape(128, 1).astype(np.float32)
    return wts, wb


def _numpy_reference(x, offset, posi_map, w1, b1, w2, b2, w_ca, bias):
    """Exact numpy port of reference.py (general-input fallback)."""
    x = np.asarray(x, np.float32)
    offset = np.asarray(offset, np.float32)
    posi_map = np.asarray(posi_map, np.float32)
    w1 = np.asarray(w1, np.float32)
    b1 = np.asarray(b1, np.float32)
    w2 = np.asarray(w2, np.float32)
    b2 = np.asarray(b2, np.float32)
    w_ca = np.asarray(w_ca, np.float32)
    bias = np.asarray(bias, np.float32)

    Bq, _, Hq, Wq = x.shape
    dx = offset[:, 0] * (2.0 / max(Wq - 1, 1)) * 0.5
    dy = offset[:, 1] * (2.0 / max(Hq - 1, 1)) * 0.5
    ys = np.linspace(-1.0, 1.0, Hq, dtype=x.dtype)
    xs = np.linspace(-1.0, 1.0, Wq, dtype=x.dtype)
    gx = xs[None, None, :] + dx
    gy = ys[None, :, None] + dy
    img = np.broadcast_to(posi_map, (Bq, posi_map.shape[1], GP, GP))

    Hp = Wp = GP
    imgT = img.transpose(0, 2, 3, 1)                      # [B, Hp, Wp, C]
    ix = np.clip((gx + 1.0) * 0.5 * (Wp - 1), 0.0, Wp - 1)
    iy = np.clip((gy + 1.0) * 0.5 * (Hp - 1), 0.0, Hp - 1)
    x0 = np.floor(ix).astype(np.int32)
    y0 = np.floor(iy).astype(np.int32)
    x1 = np.minimum(x0 + 1, Wp - 1)
    y1 = np.minimum(y0 + 1, Hp - 1)
    wx = (ix - x0.astype(ix.dtype))[..., None]
    wy = (iy - y0.astype(iy.dtype))[..., None]
    bb = np.arange(Bq)[:, None, None]
    v00 = imgT[bb, y0, x0]
    v01 = imgT[bb, y0, x1]
    v10 = imgT[bb, y1, x0]
    v11 = imgT[bb, y1, x1]
    top = v00 * (1 - wx) + v01 * wx
    bot = v10 * (1 - wx) + v11 * wx
    up = (top * (1 - wy) + bot * wy).transpose(0, 3, 1, 2)  # [B, 16, H, W]

    h = np.maximum(np.einsum('oc,bchw->bohw', w1, up) + b1[None, :, None, None], 0.0)
    weights = np.einsum('oc,bchw->bohw', w2, h) + b2[None, :, None, None]
    weights = weights.reshape(Bq, OC, KK * KK, Hq, Wq)
    x_adapt = np.einsum('oc,bchw->bohw', w_ca, x)
    xp = np.pad(x_adapt, ((0, 0), (0, 0), (1, 1), (1, 1)))
    patches = np.stack(
        [xp[:, :, i:i + Hq, j:j + Wq] for i in range(KK) for j in range(KK)],
        axis=2,
    )
    out = (weights * patches).sum(axis=2) + bias
    return out.astype(np.float32)


def _unshard_quad(arr):
    """[2, 128, NITER, 4, W] bf16 device layout -> [OC, H, W] f32."""
    a = np.asarray(arr, np.float32).reshape(2, 2, OC, NITER, 4, W)
    # dims: [half, h4, o, j, r, w] -> row = half*64 + j*8 + h4*4 + r
    return np.ascontiguousarray(
        a.transpose(2, 0, 3, 1, 4, 5).reshape(OC, H, W)
    )


def kernel(**inputs):
    global _cached_nc, _cached_variant, last_results
    x = np.ascontiguousarray(np.asarray(inputs["x"], np.float32))
    posi_map = np.asarray(inputs["posi_map"], np.float32)

    per_ch = posi_map.reshape(posi_map.shape[0] * posi_map.shape[1], -1)
    if not np.all(per_ch == per_ch[:, :1]):
        # general (spatially varying posi_map) fallback: exact numpy port
        return _numpy_reference(**{k: inputs[k] for k in (
            "x", "offset", "posi_map", "w1", "b1", "w2", "b2", "w_ca", "bias")})

    wfull, bias_vec = _host_conv_weights(
        posi_map, inputs["w1"], inputs["b1"], inputs["w2"], inputs["b2"],
        inputs["w_ca"], inputs["bias"],
    )
    wts, wb = _pack_quad(wfull, bias_vec)

    warm = int(os.environ.get("BASS_WARM_MMS", "11"))
    variant = f"quad_w{warm}"
    if _cached_nc is None or _cached_variant != variant:
        _cached_nc = _build_conv_nc_quad(warm_mms=warm)
        _cached_variant = variant

    import ml_dtypes

    xpad = np.pad(x, ((0, 0), (0, 0), (1, 1), (1, 1))).astype(ml_dtypes.bfloat16)
    wts16 = wts.astype(ml_dtypes.bfloat16)
    in_maps = [{"x": xpad[i], "wts": wts16, "wb": wb} for i in range(NCORES)]
    trace = os.environ.get("BASS_KERNEL_TRACE", "0") == "1"
    if trace:
        _ensure_ntff_hook()
    res = run_bass_kernel_spmd(
        _cached_nc, in_maps, list(range(NCORES)), trace=trace
    )
    last_results = res
    out = np.stack(
        [_unshard_quad(res.results[i]["out"]) for i in range(NCORES)],
        axis=0,
    )
    return out
